# revision 47
# baseline (speedup 1.0000x reference)
"""Trainium2 Bass kernel for nn_AdaptivePoolingClassifier (8 NeuronCores).

Math: the reference MLP is linear up to its single ReLU, so W1..W3 fold
into one 128x128 matrix on the host:
    h   = relu(x @ Wc^T + bc)       Wc = W3 W2 W1 ; bc = W3(W2 b1+b2)+b3
    p   = h @ W4^T + b4
    out = sum_n p * softmax(alpha*p, axis=1)

Device computes pt = h @ (diag(alpha) W4)^T = alpha*(p - b4) for every
row (rows sharded 8 ways) and streams pt back to DRAM; the host finishes
the softmax pooling in f64 (num/den sums over rows) exactly as it
already finishes the fold / bias algebra.  The softmax weights are
invariant to the per-column constant alpha*b4 shift.

Device schedule (v6) — measured-engine-rate driven:
  - x host-transposed to [128(feat), rows] fp8e4; weights bf16
    (mixed-dtype matmul, rel err ~1.8e-3 after 200k-row pooling).
  - All x DMAs issued up front into persistent SBUF tiles
    (24.5KB/partition): cst on the gpsimd ring concurrently with
    ramping-size x groups FIFO on the sync ring (per-queue FIFO =
    first groups complete first; small early groups cut the
    first-tile data latency to ~10us incl the ~7us engine preamble).
  - PE warms up on memset junk from ~7.5us (7x512-col matmuls, no
    data deps) so the ~3us DVFS ramp overlaps the initial DMA; the
    first real L1 runs near full clock.
  - Steady state: 4-tile L1 blocks (1024 cols each, 2x512 matmuls
    into 2-bank psum tiles, bufs=3) followed by chunk trains (up to
    6 tiles x 8 L4 chunk matmuls; h-chunk stationary [128,128] bf16,
    w4at moving 5 cols; dense ldweights pipeline ~25ns/chunk).
    Whole-tile relus alternate ACT/DVE (1114ns vs 1224ns per 1024
    cols measured); ramp-era tiles 0-2 and tail tiles 20-23 split
    across both engines so the end-of-run relu load is balanced
    (both engines ~2.8us) instead of serializing 3.6us on ACT.  Trace analysis showed chunk
    trains stall ONLY on DVE-relu tiles (~400-900ns each), so DVE
    tiles get one extra block of chunk lag (pops stay in order ->
    pt destinations stay contiguous, no extra DMAs).
  - pt accumulates in psum [128, 52, 5] batches ending at chunk
    (48, 96, 144, 188, 196); copies alternate engines; ptb bufs=3 so
    the final small batch's copy never WAR-waits on the prior
    batch's in-flight DMA.
"""

import numpy as np
import ml_dtypes

from concourse import bacc, mybir, tile
from concourse.bass_utils import run_bass_kernel_spmd

N_CORES = 8
N_ROWS = 200000
F = 128
OUT = 5

ROWS_PAD = 200704            # 8 * 25088
RPC = ROWS_PAD // N_CORES    # rows per core = 25088
T0 = 512                     # prologue tile
TILE = 1024                  # steady tile (2 psum banks)
N_TILES = (RPC - T0) // TILE  # 24
CHUNK = 128
N_CHUNKS = RPC // CHUNK      # 196
SLOTS = 52                   # max pt chunks per psum batch
BATCH_ENDS = (48, 96, 144, 188, 196)
BATCH_ENGINE = ("act", "dve", "act", "act", "dve")
BATCH_QUEUE = ("sync", "sync", "sync", "sync", "sync")
# ramping x DMA groups: cst+xg0 on gpsimd (earliest queue), rest FIFO
# on sync; early groups small so the first tiles never starve
XG_SIZES = (T0 // 2, T0 // 2, TILE, TILE, 2 * TILE, 3 * TILE,
            4 * TILE, 5 * TILE, 8 * TILE)
# tiles on the DVE relu path (rest on ACT); first three and last two
# split across both engines (latency-critical)
DVE_TILES = frozenset((5, 7, 9, 11, 13, 15, 17, 19))
SPLIT_TILES = frozenset((-1, 0, 1, 2, 20, 21, 22, 23))

F32 = mybir.dt.float32
BF16 = mybir.dt.bfloat16
FP8 = mybir.dt.float8e4
AF = mybir.ActivationFunctionType
ALU = mybir.AluOpType


def build_bass(has_bias=False):
    nc = bacc.Bacc()

    # wct separate so its rows are contiguous (faster LDWEIGHTS)
    wct_ext = nc.declare_dram_parameter("wct", [F, F], BF16, isOutput=False)
    CONST_COLS = (OUT + 1) if has_bias else OUT
    cst_ext = nc.declare_dram_parameter(
        "cst", [F, CONST_COLS], BF16, isOutput=False
    )
    xg_ext = [
        nc.declare_dram_parameter(f"xg{i}", [F, w], FP8, isOutput=False)
        for i, w in enumerate(XG_SIZES)
    ]
    pt_ext = nc.declare_dram_parameter(
        "pt", [F, N_CHUNKS, OUT], F32, isOutput=True
    )

    with tile.TileContext(nc) as tc:
        with (
            tc.tile_pool(name="scratch", bufs=1) as scratch,
            tc.tile_pool(name="xin", bufs=1) as xin,
            tc.tile_pool(name="hbuf", bufs=8) as hbuf,
            tc.tile_pool(name="ptb", bufs=3) as ptb,
            tc.tile_pool(name="ps_h", bufs=3, space="PSUM") as ps_h,
            tc.tile_pool(name="ps_p", bufs=2, space="PSUM") as ps_p,
        ):
            # --- junk memsets first on gpsimd (earliest user queue) ------
            junk_w = scratch.tile([F, CHUNK], BF16)
            junk_x = scratch.tile([F, 512], FP8)
            junk_o = scratch.tile([F, 8], BF16)


            # --- upfront DMA issue, ALL on sync: per-queue FIFO means
            # cst/xg0 complete first at full aggregate DMA bandwidth ------
            wctt = scratch.tile([F, F], BF16)
            cstt = scratch.tile([F, CONST_COLS], BF16)
            xg = [xin.tile([F, w], FP8, tag=f"xg{i}", name=f"xg{i}")
                  for i, w in enumerate(XG_SIZES)]
            nc.vector.memset(junk_w[:], 1.0)
            nc.vector.memset(junk_x[:], 1.0)
            nc.gpsimd.dma_start(out=wctt[:], in_=wct_ext[:])
            nc.gpsimd.dma_start(out=cstt[:], in_=cst_ext[:])
            for i in range(len(XG_SIZES)):
                nc.sync.dma_start(out=xg[i][:], in_=xg_ext[i][:])

            wct = wctt[:]
            w4at = cstt[:, :OUT]
            bc = None
            if has_bias:
                bc = scratch.tile([F, 1], F32)

            # --- engine warmups (gated only on the vector memsets) -------
            pw = ps_h.tile([F, TILE], F32, tag="hp", name="pw")
            # preload ACT relu table during the DMA wait
            nc.scalar.activation(junk_o[:], junk_w[:, :8], AF.Relu)
            for r in range(7):
                nc.tensor.matmul(
                    pw[:, :512], junk_w, junk_x[:], start=True,
                    stop=True, skip_group_check=True,
                )
            if has_bias:
                nc.vector.tensor_copy(bc[:], cstt[:, OUT : OUT + 1])

            state = {"chunk": 0, "pp": None, "bi": 0, "dests": []}
            hbufs = []  # per tile: (htile, n_chunks)

            def act_relu(dst, src):
                if has_bias:
                    nc.scalar.activation(dst, src, AF.Relu, bias=bc[:], scale=1.0)
                else:
                    nc.scalar.activation(dst, src, AF.Relu)

            def dve_relu(dst, src):
                if has_bias:
                    nc.vector.tensor_scalar(dst, src, bc[:], 0.0, ALU.add, ALU.max)
                else:
                    nc.vector.tensor_scalar_max(dst, src, 0.0)

            def do_l1(ti, rhs, width):
                hp = ps_h.tile([F, TILE], F32, tag="hp", name="hp")
                for c in range(0, width, 512):
                    nc.tensor.matmul(
                        hp[:, c : c + 512], wct, rhs[:, c : c + 512],
                        start=True, stop=True,
                    )
                ht = hbuf.tile([F, TILE], BF16, tag="ht")
                if ti in SPLIT_TILES:
                    act_relu(ht[:, : width // 2], hp[:, : width // 2])
                    dve_relu(ht[:, width // 2 : width], hp[:, width // 2 : width])
                elif ti in DVE_TILES:
                    dve_relu(ht[:, :width], hp[:, :width])
                else:
                    act_relu(ht[:, :width], hp[:, :width])
                base = 0 if ti < 0 else T0 // CHUNK + ti * (TILE // CHUNK)
                hbufs.append((ht, width // CHUNK, base))

            def flush_batch():
                dests = state["dests"]
                n = len(dests)
                bi = state["bi"]
                pts = ptb.tile([F, SLOTS, OUT], F32, tag="pts")
                if BATCH_ENGINE[bi] == "dve":
                    nc.vector.tensor_copy(pts[:, :n, :], state["pp"][:, :n, :])
                else:
                    nc.scalar.activation(
                        pts[:, :n, :], state["pp"][:, :n, :], AF.Copy,
                    )
                # one DMA per contiguous destination-chunk run; the
                # second-to-last batch issues from the idle gpsimd queue
                # so the two final DMA gens run in parallel
                eng = nc.gpsimd if BATCH_QUEUE[bi] == "gpsimd" else nc.sync
                s0 = 0
                while s0 < n:
                    s1 = s0 + 1
                    while s1 < n and dests[s1] == dests[s1 - 1] + 1:
                        s1 += 1
                    eng.dma_start(
                        out=pt_ext[:, dests[s0] : dests[s1 - 1] + 1, :],
                        in_=pts[:, s0:s1, :],
                    )
                    s0 = s1
                state["dests"] = []
                state["bi"] = bi + 1

            def do_l4(ti):
                ht, n_ch, base = hbufs[ti]
                for j in range(n_ch):
                    s = len(state["dests"])
                    if s == 0:
                        state["pp"] = ps_p.tile(
                            [F, SLOTS, OUT], F32, tag="pp", name="pp"
                        )
                    nc.tensor.matmul(
                        state["pp"][:, s, :],
                        ht[:, j * CHUNK : (j + 1) * CHUNK], w4at,
                        start=True, stop=True,
                    )
                    state["dests"].append(base + j)
                    state["chunk"] += 1
                    if state["chunk"] in BATCH_ENDS:
                        flush_batch()

            # prologue tile (hbufs[0]); steady tile t -> hbufs[t+1]
            bounds = []
            b = 0
            for w in XG_SIZES:
                bounds.append((b, b + w))
                b += w

            def xg_slice(c0, width):
                for g, (lo, hi) in enumerate(bounds):
                    if lo <= c0 and c0 + width <= hi:
                        return xg[g][:, c0 - lo : c0 - lo + width]
                raise AssertionError(f"tile [{c0}, {c0+width}) crosses groups")

            # 4-tile L1 blocks + lag-2 chunk trains: fewer block<->train
            # transitions, PE stays dense through the DVFS ramp
            # prologue: two 256-col matmuls so the first starts as soon
            # as the first half of xg0 lands on the cold DMA ring
            hp0 = ps_h.tile([F, TILE], F32, tag="hp", name="hp0")
            nc.tensor.matmul(hp0[:, 0:256], wct, xg[0][:], start=True,
                             stop=True)
            nc.tensor.matmul(hp0[:, 256:512], wct, xg[1][:], start=True,
                             stop=True)
            ht0 = hbuf.tile([F, TILE], BF16, tag="ht", name="ht0")
            act_relu(ht0[:, 0:256], hp0[:, 0:256])
            dve_relu(ht0[:, 256:512], hp0[:, 256:512])
            hbufs.append((ht0, T0 // CHUNK, 0))
            pending = [0]                # hbufs indices awaiting chunks
            for b in range(0, N_TILES, 4):
                order = range(b, b + 4)
                for t in order:
                    do_l1(t, xg_slice(T0 + t * TILE, TILE), TILE)
                pending.extend((b + 1, b + 2, b + 3, b + 4))
                popped = 0
                while pending and popped < 6:
                    pos = pending[0]
                    # DVE relus are ~800ns slower: give those tiles one
                    # extra block of lag so their chunks never wait
                    lim = b if (pos - 1) in DVE_TILES else b + 2
                    if pos > lim:
                        break
                    do_l4(pending.pop(0))
                    popped += 1
            for k in pending:
                do_l4(k)

    nc.finalize()
    return nc


_CACHED = {}
TRACE = False
LAST = {}


def kernel(x, W1, b1, W2, b2, W3, b3, W4, b4, alpha):
    f64 = np.float64
    x2 = np.asarray(x, np.float32).reshape(N_ROWS, F)
    W1, b1, W2, b2, W3, b3, W4, b4, alpha = [
        np.asarray(a, f64) for a in (W1, b1, W2, b2, W3, b3, W4, b4, alpha)
    ]

    # fold the linear layers (exact in f64)
    Wc = W3 @ W2 @ W1
    bc = W3 @ (W2 @ b1 + b2) + b3
    alpha_safe = np.where(np.abs(alpha) < 1e-12, 1e-12, alpha)
    W4a = alpha_safe[:, None] * W4

    # pad rows to 8*25088 with zeros; pad rows dropped after the gather
    n_pad = ROWS_PAD - N_ROWS
    xp = np.concatenate([x2, np.zeros((n_pad, F), np.float32)], axis=0)
    xT = np.ascontiguousarray(xp.T).astype(ml_dtypes.float8_e4m3fn)

    has_bias = bool(np.any(bc != 0.0))
    key = ("nc", has_bias)
    if key not in _CACHED:
        _CACHED[key] = build_bass(has_bias)
    nc = _CACHED[key]

    wct_np = np.ascontiguousarray(Wc.T).astype(ml_dtypes.bfloat16)
    w4at_np = np.ascontiguousarray(W4a.T).astype(ml_dtypes.bfloat16)
    parts_list = [w4at_np]
    if has_bias:
        parts_list.append(
            bc.reshape(F, 1).astype(np.float32).astype(ml_dtypes.bfloat16)
        )
    consts_np = np.ascontiguousarray(np.concatenate(parts_list, axis=1))

    bounds = np.cumsum((0,) + XG_SIZES)
    in_maps = []
    for c in range(N_CORES):
        shard = xT[:, c * RPC : (c + 1) * RPC]
        m = {"cst": consts_np, "wct": wct_np}
        for i in range(len(XG_SIZES)):
            m[f"xg{i}"] = np.ascontiguousarray(shard[:, bounds[i] : bounds[i + 1]])
        in_maps.append(m)

    res = run_bass_kernel_spmd(
        nc, in_maps, core_ids=list(range(N_CORES)), trace=TRACE
    )
    LAST["res"] = res

    # gather pt: per core [F(part=row-in-chunk), N_CHUNKS, OUT]
    pts = np.stack([np.asarray(r["pt"], np.float32) for r in res.results])
    # rows order: (core, chunk, partition)
    pt = pts.transpose(0, 2, 1, 3).reshape(ROWS_PAD, OUT).astype(f64)
    pt = pt[:N_ROWS]

    # host softmax pooling in f64:  out_o = sum pt*e^pt / (alpha*sum e^pt) + b4
    m = pt.max(axis=0)
    e = np.exp(pt - m)
    den = e.sum(axis=0)
    num = (pt * e).sum(axis=0)
    out = num / (alpha_safe * den) + b4
    return out[None, :].astype(np.float32)


# revision 48
# speedup vs baseline: 1.0461x; 1.0461x over previous
"""Trainium2 Bass kernel for nn_AdaptivePoolingClassifier (8 NeuronCores).

Math: the reference MLP is linear up to its single ReLU, so W1..W3 fold
into one 128x128 matrix on the host:
    h   = relu(x @ Wc^T + bc)       Wc = W3 W2 W1 ; bc = W3(W2 b1+b2)+b3
    p   = h @ W4^T + b4
    out = sum_n p * softmax(alpha*p, axis=1)

Device computes pt = h @ (diag(alpha) W4)^T = alpha*(p - b4) for every
row (rows sharded 8 ways) and streams pt back to DRAM; the host finishes
the softmax pooling in f64 (num/den sums over rows) exactly as it
already finishes the fold / bias algebra.  The softmax weights are
invariant to the per-column constant alpha*b4 shift.

Device schedule (v6) — measured-engine-rate driven:
  - x host-transposed to [128(feat), rows] fp8e4; weights bf16
    (mixed-dtype matmul, rel err ~1.8e-3 after 200k-row pooling).
  - All x DMAs issued up front into persistent SBUF tiles
    (24.5KB/partition): cst on the gpsimd ring concurrently with
    ramping-size x groups FIFO on the sync ring (per-queue FIFO =
    first groups complete first; small early groups cut the
    first-tile data latency to ~10us incl the ~7us engine preamble).
  - PE warms up on memset junk from ~7.5us (7x512-col matmuls, no
    data deps) so the ~3us DVFS ramp overlaps the initial DMA; the
    first real L1 runs near full clock.
  - Steady state: 4-tile L1 blocks (1024 cols each, 2x512 matmuls
    into 2-bank psum tiles, bufs=3) followed by chunk trains (up to
    6 tiles x 8 L4 chunk matmuls; h-chunk stationary [128,128] bf16,
    w4at moving 5 cols; dense ldweights pipeline ~25ns/chunk).
    Whole-tile relus alternate ACT/DVE (1114ns vs 1224ns per 1024
    cols measured); ramp-era tiles 0-2 and tail tiles 20-23 split
    across both engines so the end-of-run relu load is balanced
    (both engines ~2.8us) instead of serializing 3.6us on ACT.  Trace analysis showed chunk
    trains stall ONLY on DVE-relu tiles (~400-900ns each), so DVE
    tiles get one extra block of chunk lag (pops stay in order ->
    pt destinations stay contiguous, no extra DMAs).
  - pt accumulates in psum [128, 52, 5] batches ending at chunk
    (48, 96, 144, 188, 196); copies alternate engines; ptb bufs=3 so
    the final small batch's copy never WAR-waits on the prior
    batch's in-flight DMA.
"""

import numpy as np
import ml_dtypes

from concourse import bacc, mybir, tile
from concourse.bass_utils import run_bass_kernel_spmd

N_CORES = 8
N_ROWS = 200000
F = 128
OUT = 5

ROWS_PAD = 200704            # 8 * 25088
RPC = ROWS_PAD // N_CORES    # rows per core = 25088
T0 = 512                     # prologue tile
TILE = 1024                  # steady tile (2 psum banks)
N_TILES = (RPC - T0) // TILE  # 24
CHUNK = 128
N_CHUNKS = RPC // CHUNK      # 196
SLOTS = 52                   # max pt chunks per psum batch
BATCH_ENDS = (48, 96, 144, 188, 196)
BATCH_ENGINE = ("act", "dve", "act", "act", "dve")
BATCH_QUEUE = ("sync", "sync", "sync", "sync", "sync")
# ramping x DMA groups: cst+xg0 on gpsimd (earliest queue), rest FIFO
# on sync; early groups small so the first tiles never starve
XG_SIZES = (T0, TILE, TILE, 2 * TILE, 3 * TILE, 4 * TILE, 5 * TILE,
            8 * TILE)
# tiles on the DVE relu path (rest on ACT); first three and last two
# split across both engines (latency-critical)
DVE_TILES = frozenset((5, 7, 9, 11, 13, 15, 17, 19))
SPLIT_TILES = frozenset((-1, 0, 1, 2, 20, 21, 22, 23))

F32 = mybir.dt.float32
BF16 = mybir.dt.bfloat16
FP8 = mybir.dt.float8e4
AF = mybir.ActivationFunctionType
ALU = mybir.AluOpType


def build_bass(has_bias=False):
    nc = bacc.Bacc()

    # wct separate so its rows are contiguous (faster LDWEIGHTS)
    wct_ext = nc.declare_dram_parameter("wct", [F, F], BF16, isOutput=False)
    CONST_COLS = (OUT + 1) if has_bias else OUT
    cst_ext = nc.declare_dram_parameter(
        "cst", [F, CONST_COLS], BF16, isOutput=False
    )
    xg_ext = [
        nc.declare_dram_parameter(f"xg{i}", [F, w], FP8, isOutput=False)
        for i, w in enumerate(XG_SIZES)
    ]
    pt_ext = nc.declare_dram_parameter(
        "pt", [F, N_CHUNKS, OUT], F32, isOutput=True
    )

    with tile.TileContext(nc) as tc:
        with (
            tc.tile_pool(name="scratch", bufs=1) as scratch,
            tc.tile_pool(name="xin", bufs=1) as xin,
            tc.tile_pool(name="hbuf", bufs=8) as hbuf,
            tc.tile_pool(name="ptb", bufs=3) as ptb,
            tc.tile_pool(name="ps_h", bufs=3, space="PSUM") as ps_h,
            tc.tile_pool(name="ps_p", bufs=2, space="PSUM") as ps_p,
        ):
            # --- junk memsets first on gpsimd (earliest user queue) ------
            junk_w = scratch.tile([F, CHUNK], BF16)
            junk_x = scratch.tile([F, 512], FP8)
            junk_o = scratch.tile([F, 8], BF16)


            # --- upfront DMA issue, ALL on sync: per-queue FIFO means
            # cst/xg0 complete first at full aggregate DMA bandwidth ------
            wctt = scratch.tile([F, F], BF16)
            cstt = scratch.tile([F, CONST_COLS], BF16)
            xg = [xin.tile([F, w], FP8, tag=f"xg{i}", name=f"xg{i}")
                  for i, w in enumerate(XG_SIZES)]
            nc.vector.memset(junk_w[:], 1.0)
            nc.vector.memset(junk_x[:], 1.0)
            nc.gpsimd.dma_start(out=wctt[:], in_=wct_ext[:])
            nc.gpsimd.dma_start(out=cstt[:], in_=cst_ext[:])
            for i in range(len(XG_SIZES)):
                nc.sync.dma_start(out=xg[i][:], in_=xg_ext[i][:])

            wct = wctt[:]
            w4at = cstt[:, :OUT]
            bc = None
            if has_bias:
                bc = scratch.tile([F, 1], F32)

            # --- engine warmups (gated only on the vector memsets) -------
            pw = ps_h.tile([F, TILE], F32, tag="hp", name="pw")
            # preload ACT relu table during the DMA wait
            nc.scalar.activation(junk_o[:], junk_w[:, :8], AF.Relu)
            for r in range(7):
                nc.tensor.matmul(
                    pw[:, :512], junk_w, junk_x[:], start=True,
                    stop=True, skip_group_check=True,
                )
            if has_bias:
                nc.vector.tensor_copy(bc[:], cstt[:, OUT : OUT + 1])

            state = {"chunk": 0, "pp": None, "bi": 0, "dests": []}
            hbufs = []  # per tile: (htile, n_chunks)

            def act_relu(dst, src):
                if has_bias:
                    nc.scalar.activation(dst, src, AF.Relu, bias=bc[:], scale=1.0)
                else:
                    nc.scalar.activation(dst, src, AF.Relu)

            def dve_relu(dst, src):
                if has_bias:
                    nc.vector.tensor_scalar(dst, src, bc[:], 0.0, ALU.add, ALU.max)
                else:
                    nc.vector.tensor_scalar_max(dst, src, 0.0)

            def do_l1(ti, rhs, width):
                hp = ps_h.tile([F, TILE], F32, tag="hp", name="hp")
                for c in range(0, width, 512):
                    nc.tensor.matmul(
                        hp[:, c : c + 512], wct, rhs[:, c : c + 512],
                        start=True, stop=True,
                    )
                ht = hbuf.tile([F, TILE], BF16, tag="ht")
                if ti in SPLIT_TILES:
                    act_relu(ht[:, : width // 2], hp[:, : width // 2])
                    dve_relu(ht[:, width // 2 : width], hp[:, width // 2 : width])
                elif ti in DVE_TILES:
                    dve_relu(ht[:, :width], hp[:, :width])
                else:
                    act_relu(ht[:, :width], hp[:, :width])
                base = 0 if ti < 0 else T0 // CHUNK + ti * (TILE // CHUNK)
                hbufs.append((ht, width // CHUNK, base))

            def flush_batch():
                dests = state["dests"]
                n = len(dests)
                bi = state["bi"]
                pts = ptb.tile([F, SLOTS, OUT], F32, tag="pts")
                if BATCH_ENGINE[bi] == "dve":
                    nc.vector.tensor_copy(pts[:, :n, :], state["pp"][:, :n, :])
                else:
                    nc.scalar.activation(
                        pts[:, :n, :], state["pp"][:, :n, :], AF.Copy,
                    )
                # one DMA per contiguous destination-chunk run; the
                # second-to-last batch issues from the idle gpsimd queue
                # so the two final DMA gens run in parallel
                eng = nc.gpsimd if BATCH_QUEUE[bi] == "gpsimd" else nc.sync
                s0 = 0
                while s0 < n:
                    s1 = s0 + 1
                    while s1 < n and dests[s1] == dests[s1 - 1] + 1:
                        s1 += 1
                    eng.dma_start(
                        out=pt_ext[:, dests[s0] : dests[s1 - 1] + 1, :],
                        in_=pts[:, s0:s1, :],
                    )
                    s0 = s1
                state["dests"] = []
                state["bi"] = bi + 1

            def do_l4(ti):
                ht, n_ch, base = hbufs[ti]
                for j in range(n_ch):
                    s = len(state["dests"])
                    if s == 0:
                        state["pp"] = ps_p.tile(
                            [F, SLOTS, OUT], F32, tag="pp", name="pp"
                        )
                    nc.tensor.matmul(
                        state["pp"][:, s, :],
                        ht[:, j * CHUNK : (j + 1) * CHUNK], w4at,
                        start=True, stop=True,
                    )
                    state["dests"].append(base + j)
                    state["chunk"] += 1
                    if state["chunk"] in BATCH_ENDS:
                        flush_batch()

            # prologue tile (hbufs[0]); steady tile t -> hbufs[t+1]
            bounds = []
            b = 0
            for w in XG_SIZES:
                bounds.append((b, b + w))
                b += w

            def xg_slice(c0, width):
                for g, (lo, hi) in enumerate(bounds):
                    if lo <= c0 and c0 + width <= hi:
                        return xg[g][:, c0 - lo : c0 - lo + width]
                raise AssertionError(f"tile [{c0}, {c0+width}) crosses groups")

            # 4-tile L1 blocks + lag-2 chunk trains: fewer block<->train
            # transitions, PE stays dense through the DVFS ramp
            do_l1(-1, xg_slice(0, T0), T0)
            pending = [0]                # hbufs indices awaiting chunks
            for b in range(0, N_TILES, 4):
                order = range(b, b + 4)
                for t in order:
                    do_l1(t, xg_slice(T0 + t * TILE, TILE), TILE)
                pending.extend((b + 1, b + 2, b + 3, b + 4))
                popped = 0
                while pending and popped < 6:
                    pos = pending[0]
                    # DVE relus are ~800ns slower: give those tiles one
                    # extra block of lag so their chunks never wait
                    lim = b if (pos - 1) in DVE_TILES else b + 2
                    if pos > lim:
                        break
                    do_l4(pending.pop(0))
                    popped += 1
            for k in pending:
                do_l4(k)

    nc.finalize()
    return nc


_CACHED = {}
TRACE = False
LAST = {}


def kernel(x, W1, b1, W2, b2, W3, b3, W4, b4, alpha):
    f64 = np.float64
    x2 = np.asarray(x, np.float32).reshape(N_ROWS, F)
    W1, b1, W2, b2, W3, b3, W4, b4, alpha = [
        np.asarray(a, f64) for a in (W1, b1, W2, b2, W3, b3, W4, b4, alpha)
    ]

    # fold the linear layers (exact in f64)
    Wc = W3 @ W2 @ W1
    bc = W3 @ (W2 @ b1 + b2) + b3
    alpha_safe = np.where(np.abs(alpha) < 1e-12, 1e-12, alpha)
    W4a = alpha_safe[:, None] * W4

    # pad rows to 8*25088 with zeros; pad rows dropped after the gather
    n_pad = ROWS_PAD - N_ROWS
    xp = np.concatenate([x2, np.zeros((n_pad, F), np.float32)], axis=0)
    xT = np.ascontiguousarray(xp.T).astype(ml_dtypes.float8_e4m3fn)

    has_bias = bool(np.any(bc != 0.0))
    key = ("nc", has_bias)
    if key not in _CACHED:
        _CACHED[key] = build_bass(has_bias)
    nc = _CACHED[key]

    wct_np = np.ascontiguousarray(Wc.T).astype(ml_dtypes.bfloat16)
    w4at_np = np.ascontiguousarray(W4a.T).astype(ml_dtypes.bfloat16)
    parts_list = [w4at_np]
    if has_bias:
        parts_list.append(
            bc.reshape(F, 1).astype(np.float32).astype(ml_dtypes.bfloat16)
        )
    consts_np = np.ascontiguousarray(np.concatenate(parts_list, axis=1))

    bounds = np.cumsum((0,) + XG_SIZES)
    in_maps = []
    for c in range(N_CORES):
        shard = xT[:, c * RPC : (c + 1) * RPC]
        m = {"cst": consts_np, "wct": wct_np}
        for i in range(len(XG_SIZES)):
            m[f"xg{i}"] = np.ascontiguousarray(shard[:, bounds[i] : bounds[i + 1]])
        in_maps.append(m)

    res = run_bass_kernel_spmd(
        nc, in_maps, core_ids=list(range(N_CORES)), trace=TRACE
    )
    LAST["res"] = res

    # gather pt: per core [F(part=row-in-chunk), N_CHUNKS, OUT]
    pts = np.stack([np.asarray(r["pt"], np.float32) for r in res.results])
    # rows order: (core, chunk, partition)
    pt = pts.transpose(0, 2, 1, 3).reshape(ROWS_PAD, OUT).astype(f64)
    pt = pt[:N_ROWS]

    # host softmax pooling in f64:  out_o = sum pt*e^pt / (alpha*sum e^pt) + b4
    m = pt.max(axis=0)
    e = np.exp(pt - m)
    den = e.sum(axis=0)
    num = (pt * e).sum(axis=0)
    out = num / (alpha_safe * den) + b4
    return out[None, :].astype(np.float32)
